# revision 13
# baseline (speedup 1.0000x reference)
"""Trainium2 Bass kernel for nn_AdvancedTrustTemporalGNN.

Strategy (8 NeuronCores, SPMD):
  - Host relabels nodes by in-degree (desc) into 128-node windows; each node's
    incoming edges occupy "slots" in its partition column, windows padded to the
    window max degree.  Edge tiles are therefore node-aligned: partition p of
    every slot tile in window k belongs to dst node (window k, p).  Segment
    max / sum / weighted-scatter become plain per-tile DVE ops.
  - Nodes (and their edge slots) are sharded across the 8 cores; the per-node
    K/V/KT table is computed sharded and shared via one AllGather per layer.
    src-side rows are fetched with indirect-DMA gathers from the gathered table.
  - All matmuls fp32 except the last layer's MoE/o_w chain (bf16) - the huge
    layer-2 attention logits (~3e4) make anything less than fp32 flip the
    softmax argmax.
  - Activations live feature-major (hT: feat x node) for matmuls; tables and
    LN statistics are produced node-major via lhsT=hT-chunk matmuls.
"""
import math
import numpy as np
from contextlib import ExitStack

import concourse.bass as bass
import concourse.bacc as bacc
import concourse.mybir as mybir
import concourse.tile as tile
from concourse.bass_utils import run_bass_kernel_spmd
from concourse.masks import make_identity

# ---------------------------------------------------------------- constants
N, E = 16384, 131072
IN_DIM, EDGE_DIM, D = 256, 128, 512
L, H, KVH = 2, 8, 2
HD = D // H            # 64
KVD = KVH * HD         # 128
G = H // KVH           # 4
NEXP = 4
LN_EPS = 1e-5
NC = 8
P = 128
NPC = N // NC          # 2048 nodes per core
NW = NPC // P          # 16 windows per core
FP32 = mybir.dt.float32
BF16 = mybir.dt.bfloat16
I32 = mybir.dt.int32
AF = mybir.ActivationFunctionType
OP = mybir.AluOpType
PI = float(np.pi)
INV2PI = float(1.0 / (2 * np.pi))
NEG_BIG = -1.0e30
# last layer MoE/o_w in bf16 (validated in simulation); set False for all-fp32
MIX_LAST_LAYER_BF16 = True


def set_size(n, e):
    """Debug helper: shrink the problem (n % 4096 == 0)."""
    global N, E, NPC, NW
    N, E = n, e
    NPC = N // NC
    NW = NPC // P


# ---------------------------------------------------------------- host prep
def _host_prep(inputs):
    x = np.ascontiguousarray(np.asarray(inputs["x"], np.float32))
    t = np.asarray(inputs["node_time"], np.float32).reshape(N)
    ea = np.ascontiguousarray(np.asarray(inputs["edge_attr"], np.float32))
    ei = np.asarray(inputs["edge_index"]).astype(np.int64)
    mem = np.asarray(inputs["mem"], np.float32)
    mem_var = np.asarray(inputs["mem_var"], np.float32)
    src, dst = ei[0], ei[1]

    deg = np.bincount(dst, minlength=N)
    order = np.argsort(-deg, kind="stable")        # old ids, degree desc
    rank = np.arange(N)
    g_of_rank = rank // P                          # global window
    c_of_rank = g_of_rank % NC                     # core
    k_of_rank = g_of_rank // NC                    # window-within-core
    p_of_rank = rank % P
    new_of_rank = c_of_rank * NPC + k_of_rank * P + p_of_rank
    old2new = np.empty(N, np.int64)
    old2new[order] = new_of_rank
    new2old = np.empty(N, np.int64)
    new2old[new_of_rank] = order

    wmax = deg[order].reshape(N // P, P).max(1)    # per global window max deg
    That = [max(1, int(wmax[NC * k:NC * k + NC].max())) for k in range(NW)]
    tile_base = np.concatenate([[0], np.cumsum(That)]).astype(np.int64)
    NT = int(tile_base[-1])
    NSLOT = NT * P

    dstn = old2new[dst]
    srcn = old2new[src]
    c_e = dstn // NPC
    k_e = (dstn % NPC) // P
    p_e = dstn % P
    eorder = np.argsort(dstn, kind="stable")
    dsts = dstn[eorder]
    j_sorted = np.arange(E) - np.searchsorted(dsts, dsts)
    j = np.empty(E, np.int64)
    j[eorder] = j_sorted

    slot = (tile_base[k_e] + j) * P + p_e          # per-edge slot within its core

    per_core = []
    for c in range(NC):
        sel = c_e == c
        sl = slot[sel]
        srcid = np.zeros((NSLOT, 1), np.int32)
        maskcat = np.zeros((NSLOT, 2), np.float32)
        maskcat[:, 0] = NEG_BIG
        eaT = np.zeros((EDGE_DIM, NSLOT), np.float32)
        srcid[sl, 0] = srcn[sel].astype(np.int32)
        maskcat[sl, 0] = 0.0
        maskcat[sl, 1] = 1.0
        eaT[:, sl] = ea[sel].T
        nodes_old = new2old[c * NPC:(c + 1) * NPC]
        blob = {
            "xT": np.ascontiguousarray(x[nodes_old].T),
            "memT": np.ascontiguousarray(mem[nodes_old].T),
            "mvarT": np.ascontiguousarray(mem_var[nodes_old].T),
            "trow": np.ascontiguousarray(t[nodes_old].reshape(1, NPC)),
            "tcol": np.ascontiguousarray(t[nodes_old].reshape(NPC, 1)),
            "eaT": eaT,
            "srcid": srcid,
            "maskcat": maskcat,
        }
        per_core.append(blob)

    # ------------------------------------------------------------- weights
    w = {}
    f32 = lambda a: np.ascontiguousarray(np.asarray(a, np.float32))
    w["in_w"] = f32(inputs["in_w"])
    w["inbT"] = f32(inputs["in_b"]).reshape(D, 1)
    w["edge_w"] = f32(inputs["edge_w"])
    w["ebT"] = f32(inputs["edge_b"]).reshape(D, 1)
    w["invfb"] = np.tile(
        (1.0 / (10000.0 ** (np.arange(0, HD, 2, dtype=np.float32) / HD))).reshape(1, HD // 2),
        (P, 1),
    )
    w["idxeps"] = (np.arange(NEXP, dtype=np.float32) * -1e-30).reshape(1, NEXP)
    w["fngT"] = f32(inputs["fn_g"]).reshape(D, 1)
    w["fnbT"] = f32(inputs["fn_b"]).reshape(D, 1)
    for l in range(L):
        q_w = f32(inputs["q_w"][l]); k_w = f32(inputs["k_w"][l]); v_w = f32(inputs["v_w"][l])
        w[f"qkvw{l}"] = np.concatenate([q_w, k_w, v_w], axis=1)          # (512, 768)
        w[f"qkvb{l}"] = np.concatenate(
            [f32(inputs["q_b"][l]), f32(inputs["k_b"][l]), f32(inputs["v_b"][l])]
        ).reshape(1, D + 2 * KVD)
        tk_w = f32(inputs["tk_w"][l])                                     # (128, 64)
        tq_w = f32(inputs["tq_w"][l])
        e_w = f32(inputs["e_w"][l])                                       # (512, 128)
        e_b = f32(inputs["e_b"][l])                                       # (128,)
        tk_w2 = tk_w[HD:]                                                 # (64, 64)
        ew2 = np.concatenate(
            [e_w[:, kv * HD:(kv + 1) * HD] @ tk_w2 for kv in range(KVH)], axis=1
        )                                                                 # (512, 128)
        w[f"eetw{l}"] = np.concatenate([e_w, ew2], axis=1)                # (512, 256)
        etb = np.concatenate(
            [e_b[kv * HD:(kv + 1) * HD] @ tk_w2 for kv in range(KVH)]
        )
        w[f"eetb{l}"] = np.concatenate([e_b, etb]).reshape(1, 2 * KVD)
        w[f"tqw{l}"] = tq_w
        w[f"tqb{l}"] = f32(inputs["tq_b"][l]).reshape(1, HD)
        w[f"tkw{l}"] = tk_w
        w[f"tkb{l}"] = f32(inputs["tk_b"][l]).reshape(1, HD)
        w[f"toutw{l}"] = f32(inputs["tout_w"][l]).reshape(1, HD)
        w[f"toutb{l}"] = f32(inputs["tout_b"][l]).reshape(1, 1)
        w[f"logtemp{l}"] = f32(inputs["log_temp"][l]).reshape(1, H)
        w[f"gatew{l}"] = f32(inputs["gate_w"][l])                         # (512, 4)
        w[f"gateb{l}"] = f32(inputs["gate_b"][l]).reshape(1, NEXP)
        bf = MIX_LAST_LAYER_BF16 and l == L - 1
        wdt = np.dtype("bfloat16") if False else None  # placeholder
        for e in range(NEXP):
            w1 = f32(inputs["exp_w1"][l][e])                              # (512, 1024)
            w2 = f32(inputs["exp_w2"][l][e])                              # (1024, 512)
            w1b = w1.reshape(4, P, 8, P).transpose(0, 2, 1, 3).reshape(32, P, P)
            w2b = w2.reshape(8, P, 4, P).transpose(0, 2, 1, 3).reshape(32, P, P)
            if bf:
                import ml_dtypes
                w1b = w1b.astype(ml_dtypes.bfloat16)
                w2b = w2b.astype(ml_dtypes.bfloat16)
            w[f"ew1_{l}_{e}"] = w1b
            w[f"ew2_{l}_{e}"] = w2b
            w[f"eb1T_{l}_{e}"] = f32(inputs["exp_b1"][l][e]).reshape(2 * D, 1)
            w[f"eb2T_{l}_{e}"] = f32(inputs["exp_b2"][l][e]).reshape(D, 1)
        ow = f32(inputs["o_w"][l])
        if bf:
            import ml_dtypes
            ow = ow.astype(ml_dtypes.bfloat16)
        w[f"ow{l}"] = ow
        w[f"obT{l}"] = f32(inputs["o_b"][l]).reshape(D, 1)
        ada_w = f32(inputs["ada_w"][l])                                   # (512, 1024)
        ada_b = f32(inputs["ada_b"][l])                                   # (1024,)
        w1bar = ada_w.sum(0)[:D]
        w2bar = ada_w.sum(0)[D:]
        A1 = 1.0 + ada_b[:D]
        w[f"adaM{l}"] = np.stack([A1, w1bar]).astype(np.float32)          # (2, 512)
        w[f"adaA{l}"] = np.stack([A1, w1bar, w2bar, ada_b[D:]]).astype(np.float32)  # (4,512)
        w[f"rgw{l}"] = f32(inputs["rg_w"][l])                             # (512, 1)
        w[f"rgb{l}"] = f32(inputs["rg_b"][l]).reshape(1, 1)

    meta = dict(That=That, tile_base=[int(v) for v in tile_base], NT=NT,
                NSLOT=NSLOT, new2old=new2old)
    return per_core, w, meta


_PROG_CACHE = {}
LAST_EXEC_NS = None


def kernel(__trace=False, **inputs) -> np.ndarray:
    global LAST_EXEC_NS
    per_core, w, meta = _host_prep(inputs)
    key = (N, E, tuple(meta["That"]))
    if key not in _PROG_CACHE:
        _PROG_CACHE[key] = _build_program(meta)
    nc, out_name = _PROG_CACHE[key]
    in_maps = []
    for c in range(NC):
        m = dict(per_core[c])
        m.update(w)
        in_maps.append(m)
    res = run_bass_kernel_spmd(nc, in_maps, core_ids=list(range(NC)), trace=__trace)
    LAST_EXEC_NS = res.exec_time_ns
    out = np.empty((N, D), np.float32)
    for c in range(NC):
        out[meta["new2old"][c * NPC:(c + 1) * NPC]] = res.results[c][out_name]
    return out


# ------------------------------------------------------------ device program
def _build_program(meta):
    That = meta["That"]
    tile_base = meta["tile_base"]
    NT = meta["NT"]
    NSLOT = meta["NSLOT"]
    TMAX = max(That)

    nc = bacc.Bacc("TRN2", target_bir_lowering=False, debug=False, num_devices=NC)

    # ---------------- DRAM I/O declarations
    xT_d = nc.dram_tensor("xT", [IN_DIM, NPC], FP32, kind="ExternalInput").ap()
    memT_d = nc.dram_tensor("memT", [D, NPC], FP32, kind="ExternalInput").ap()
    mvarT_d = nc.dram_tensor("mvarT", [D, NPC], FP32, kind="ExternalInput").ap()
    trow_d = nc.dram_tensor("trow", [1, NPC], FP32, kind="ExternalInput").ap()
    tcol_d = nc.dram_tensor("tcol", [NPC, 1], FP32, kind="ExternalInput").ap()
    eaT_d = nc.dram_tensor("eaT", [EDGE_DIM, NSLOT], FP32, kind="ExternalInput").ap()
    srcid_d = nc.dram_tensor("srcid", [NSLOT, 1], I32, kind="ExternalInput").ap()
    maskcat_d = nc.dram_tensor("maskcat", [NSLOT, 2], FP32, kind="ExternalInput").ap()

    wd = {}
    def win(name, shape, dt=FP32):
        wd[name] = nc.dram_tensor(name, list(shape), dt, kind="ExternalInput").ap()
        return wd[name]

    win("in_w", (IN_DIM, D)); win("inbT", (D, 1))
    win("edge_w", (EDGE_DIM, D)); win("ebT", (D, 1))
    win("invfb", (P, HD // 2)); win("idxeps", (1, NEXP))
    win("fngT", (D, 1)); win("fnbT", (D, 1))
    for l in range(L):
        bf = MIX_LAST_LAYER_BF16 and l == L - 1
        WDT = BF16 if bf else FP32
        win(f"qkvw{l}", (D, D + 2 * KVD)); win(f"qkvb{l}", (1, D + 2 * KVD))
        win(f"eetw{l}", (D, 2 * KVD)); win(f"eetb{l}", (1, 2 * KVD))
        win(f"tqw{l}", (2 * HD, HD)); win(f"tqb{l}", (1, HD))
        win(f"tkw{l}", (2 * HD, HD)); win(f"tkb{l}", (1, HD))
        win(f"toutw{l}", (1, HD)); win(f"toutb{l}", (1, 1)); win(f"logtemp{l}", (1, H))
        win(f"gatew{l}", (D, NEXP)); win(f"gateb{l}", (1, NEXP))
        for e in range(NEXP):
            win(f"ew1_{l}_{e}", (32, P, P), WDT)
            win(f"ew2_{l}_{e}", (32, P, P), WDT)
            win(f"eb1T_{l}_{e}", (2 * D, 1)); win(f"eb2T_{l}_{e}", (D, 1))
        win(f"ow{l}", (D, D), WDT); win(f"obT{l}", (D, 1))
        win(f"adaM{l}", (2, D)); win(f"adaA{l}", (4, D))
        win(f"rgw{l}", (D, 1)); win(f"rgb{l}", (1, 1))

    EHT_d = nc.dram_tensor("EHT", [4 * NT * P, P], FP32, kind="Internal").ap()
    EET_d = nc.dram_tensor("EET", [NSLOT, 2 * KVD], FP32, kind="Internal").ap()
    ccin_d = nc.dram_tensor("ccin", [NPC, D + 2 * KVD], FP32, kind="Internal").ap()
    ccout_d = nc.dram_tensor("ccout", [N, D + 2 * KVD], FP32, kind="Internal",
                             addr_space="Shared").ap()
    outp_d = nc.dram_tensor("outp", [NPC, D], FP32, kind="ExternalOutput").ap()

    TB = tile_base
    X = mybir.AxisListType.X

    def kgd(ap):
        return ap.rearrange("p (kv g d) -> p kv g d", kv=KVH, g=G, d=HD)

    def kvd(ap):
        return ap.rearrange("p (kv d) -> p kv d", kv=KVH).unsqueeze(2).to_broadcast(
            [P, KVH, G, HD])

    def hdv(ap):
        return ap.rearrange("p (h d) -> p h d", h=H)

    hT_d = nc.dram_tensor("hTd", [D, NPC], FP32, kind="Internal").ap()
    mT_d = nc.dram_tensor("mTd", [D, NPC], FP32, kind="Internal").ap()
    wkT_d = nc.dram_tensor("wkTd", [NEXP, NPC], FP32, kind="Internal").ap()

    with ExitStack() as ctx:
        tc = ctx.enter_context(tile.TileContext(nc))
        cpool = ctx.enter_context(tc.tile_pool(name="const", bufs=1))
        pers = ctx.enter_context(tc.tile_pool(name="pers", bufs=1))
        wp = ctx.enter_context(tc.tile_pool(name="wp", bufs=1))

        # ---------------- constants
        ones_row = cpool.tile([1, P], FP32)
        nc.gpsimd.memset(ones_row[:], 1.0)
        ones_col = cpool.tile([P, 1], FP32)
        nc.gpsimd.memset(ones_col[:], 1.0)
        ident = cpool.tile([P, P], FP32)
        make_identity(nc, ident[:])
        epsb = cpool.tile([1, 1], FP32)
        nc.gpsimd.memset(epsb[:], LN_EPS)

        # ---------------- persistent SBUF
        aT = [pers.tile([P, NPC], FP32, tag=f"aT{fc}", name=f"aT{fc}") for fc in range(4)]
        cosb = pers.tile([P, NW * 32], FP32, tag="cosb")
        sinb = pers.tile([P, NW * 32], FP32, tag="sinb")
        silu_row = pers.tile([1, NPC], FP32, tag="silurow")
        idxeps_bc = pers.tile([P, NEXP], FP32, tag="idxepsbc")

        # ---------------- trig / silu / idxeps / M~T setup
        with tc.tile_pool(name="setp", bufs=2) as setp:
            tcol_sb = cpool.tile([P, NW], FP32)
            nc.sync.dma_start(tcol_sb[:], tcol_d.rearrange("(i p) one -> p (i one)", p=P))
            invf_t = cpool.tile([P, HD // 2], FP32)
            nc.sync.dma_start(invf_t[:], wd["invfb"][:])
            trow_t = setp.tile([1, NPC], FP32, tag="trowt")
            nc.sync.dma_start(trow_t[:], trow_d[:])
            nc.scalar.activation(silu_row[:], trow_t[:], AF.Silu)
            idxeps_t = setp.tile([1, NEXP], FP32, tag="idxepst")
            nc.sync.dma_start(idxeps_t[:], wd["idxeps"][:])
            nc.gpsimd.partition_broadcast(idxeps_bc[:], idxeps_t[:])
            for i in range(NW):
                fr = setp.tile([P, 32], FP32, tag="fr")
                nc.vector.tensor_scalar(out=fr[:], in0=invf_t[:],
                                        scalar1=tcol_sb[:, i:i + 1], scalar2=None,
                                        op0=OP.mult)
                for dstbuf, shift in ((sinb, 0.0), (cosb, PI / 2)):
                    g = setp.tile([P, 32], FP32, tag="frg")
                    nc.vector.tensor_scalar(out=g[:], in0=fr[:], scalar1=INV2PI,
                                            scalar2=shift * INV2PI, op0=OP.mult,
                                            op1=OP.add)
                    ki = setp.tile([P, 32], I32, tag="ki")
                    nc.vector.tensor_copy(ki[:], g[:])
                    kf = setp.tile([P, 32], FP32, tag="kf")
                    nc.vector.tensor_copy(kf[:], ki[:])
                    if shift:
                        frs = setp.tile([P, 32], FP32, tag="frs")
                        nc.vector.tensor_scalar(out=frs[:], in0=fr[:], scalar1=shift,
                                                scalar2=None, op0=OP.add)
                    else:
                        frs = fr
                    r = setp.tile([P, 32], FP32, tag="rr")
                    nc.vector.scalar_tensor_tensor(out=r[:], in0=kf[:],
                                                   scalar=float(-2 * PI), in1=frs[:],
                                                   op0=OP.mult, op1=OP.add)
                    nc.scalar.activation(dstbuf[:, i * 32:(i + 1) * 32], r[:], AF.Sin)
            # M~T = memT * 1/(1+sqrt(mvarT)) -> DRAM (layer-independent)
            for fc in range(4):
                for nh in range(NPC // 512):
                    nsl = slice(nh * 512, (nh + 1) * 512)
                    mv = setp.tile([P, 512], FP32, tag="mtmv")
                    nc.sync.dma_start(mv[:], mvarT_d[fc * P:(fc + 1) * P, nsl])
                    mr = setp.tile([P, 512], FP32, tag="mtmr")
                    nc.sync.dma_start(mr[:], memT_d[fc * P:(fc + 1) * P, nsl])
                    nc.scalar.activation(mv[:], mv[:], AF.Sqrt)
                    nc.vector.tensor_scalar(out=mv[:], in0=mv[:], scalar1=1.0,
                                            scalar2=None, op0=OP.add)
                    rcp = setp.tile([P, 512], FP32, tag="mtrc")
                    nc.vector.reciprocal(rcp[:], mv[:])
                    nc.vector.tensor_mul(mr[:], mr[:], rcp[:])
                    nc.sync.dma_start(mT_d[fc * P:(fc + 1) * P, nsl], mr[:])

        def emit_rope(dst_ap, src_ap, i, nh, pool):
            rot = pool.tile([P, nh * HD], FP32, tag=f"rot{nh}", name=f"rot{nh}")
            sv = src_ap.rearrange("p (h f two) -> p h f two", h=nh, f=32, two=2)
            rv = rot[:].rearrange("p (h f two) -> p h f two", h=nh, f=32, two=2)
            nc.vector.tensor_scalar(out=rv[:, :, :, 0:1], in0=sv[:, :, :, 1:2],
                                    scalar1=-1.0, scalar2=None, op0=OP.mult)
            nc.vector.tensor_copy(rv[:, :, :, 1:2], sv[:, :, :, 0:1])
            cosv = cosb[:, i * 32:(i + 1) * 32].unsqueeze(1).unsqueeze(3).to_broadcast(
                [P, nh, 32, 2])
            sinv = sinb[:, i * 32:(i + 1) * 32].unsqueeze(1).unsqueeze(3).to_broadcast(
                [P, nh, 32, 2])
            t1 = pool.tile([P, nh * HD], FP32, tag=f"rpt{nh}", name=f"rpt{nh}")
            t1v = t1[:].rearrange("p (h f two) -> p h f two", h=nh, f=32, two=2)
            dv = dst_ap.rearrange("p (h f two) -> p h f two", h=nh, f=32, two=2)
            nc.vector.tensor_tensor(out=t1v, in0=sv, in1=cosv, op=OP.mult)
            nc.vector.tensor_tensor(out=rv, in0=rv, in1=sinv, op=OP.mult)
            nc.vector.tensor_tensor(out=dv, in0=t1v, in1=rv, op=OP.add)

        # ---------------- preamble: hT -> DRAM
        with tc.tile_pool(name="prep", bufs=2) as prep, \
             tc.tile_pool(name="psh", bufs=2, space="PSUM") as psh:
            inw_t = [wp.tile([P, D], FP32, tag=f"inw{k}", name=f"inw{k}")
                     for k in range(IN_DIM // P)]
            for k in range(IN_DIM // P):
                nc.sync.dma_start(inw_t[k][:], wd["in_w"][k * P:(k + 1) * P, :])
            inb_t = wp.tile([P, 4], FP32, tag="inbT")
            nc.sync.dma_start(inb_t[:], wd["inbT"].rearrange("(c p) one -> p (c one)", p=P))
            for nh in range(NPC // 512):
                xts = []
                for k in range(IN_DIM // P):
                    xt = prep.tile([P, 512], FP32, tag=f"xt{k}", name=f"xt{k}")
                    nc.sync.dma_start(xt[:], xT_d[k * P:(k + 1) * P,
                                                  nh * 512:(nh + 1) * 512])
                    xts.append(xt)
                for fc in range(4):
                    hp = psh.tile([P, 512], FP32, space="PSUM", tag="hps")
                    for k in range(IN_DIM // P):
                        nc.tensor.matmul(hp[:], lhsT=inw_t[k][:, fc * P:(fc + 1) * P],
                                         rhs=xts[k][:], start=(k == 0),
                                         stop=(k == IN_DIM // P - 1))
                    hsb = prep.tile([P, 512], FP32, tag="hsb")
                    nc.vector.tensor_scalar(out=hsb[:], in0=hp[:],
                                            scalar1=inb_t[:, fc:fc + 1], scalar2=None,
                                            op0=OP.add)
                    nc.sync.dma_start(hT_d[fc * P:(fc + 1) * P,
                                           nh * 512:(nh + 1) * 512], hsb[:])

        # ---------------- preamble: edge_hT blocks -> EHT
        with tc.tile_pool(name="pree", bufs=2) as pree, \
             tc.tile_pool(name="pse", bufs=2, space="PSUM") as pse:
            edw_t = wp.tile([P, D], FP32, tag="edw")
            nc.sync.dma_start(edw_t[:], wd["edge_w"][:])
            eb_t = wp.tile([P, 4], FP32, tag="ebT")
            nc.sync.dma_start(eb_t[:], wd["ebT"].rearrange("(c p) one -> p (c one)", p=P))
            c0 = 0
            while c0 < NT:
                cw = min(4, NT - c0)
                wc = cw * P
                eat = pree.tile([P, 512], FP32, tag="eat")
                nc.sync.dma_start(eat[:, :wc], eaT_d[:, c0 * P:c0 * P + wc])
                for fc in range(4):
                    ep = pse.tile([P, 512], FP32, space="PSUM", tag="ehp")
                    nc.tensor.matmul(ep[:, :wc], lhsT=edw_t[:, fc * P:(fc + 1) * P],
                                     rhs=eat[:, :wc], start=True, stop=True)
                    esb = pree.tile([P, 512], FP32, tag="esb")
                    nc.vector.tensor_scalar(out=esb[:, :wc], in0=ep[:, :wc],
                                            scalar1=eb_t[:, fc:fc + 1], scalar2=None,
                                            op0=OP.add)
                    for jj in range(cw):
                        st = c0 + jj
                        nc.sync.dma_start(
                            EHT_d[(fc * NT + st) * P:(fc * NT + st + 1) * P, :],
                            esb[:, jj * P:(jj + 1) * P])
                c0 += cw

        # ================= layers
        for l in range(L):
            bf = MIX_LAST_LAYER_BF16 and l == L - 1
            WDT = BF16 if bf else FP32

            qkvw_t = []
            for fc in range(4):
                qt_ = wp.tile([P, D + 2 * KVD], FP32, tag=f"qkvw{fc}", name=f"qkvw{fc}")
                nc.sync.dma_start(qt_[:], wd[f"qkvw{l}"][fc * P:(fc + 1) * P, :])
                qkvw_t.append(qt_)
            qkvb_t = wp.tile([1, D + 2 * KVD], FP32, tag="qkvb")
            nc.sync.dma_start(qkvb_t[:], wd[f"qkvb{l}"][:])
            tqw_t = wp.tile([P, HD], FP32, tag="tqw")
            nc.sync.dma_start(tqw_t[:], wd[f"tqw{l}"][:])
            tkw_t = wp.tile([P, HD], FP32, tag="tkw")
            nc.sync.dma_start(tkw_t[:], wd[f"tkw{l}"][:])
            tqb_t = wp.tile([1, HD], FP32, tag="tqb")
            nc.sync.dma_start(tqb_t[:], wd[f"tqb{l}"][:])
            tkb_t = wp.tile([1, HD], FP32, tag="tkb")
            nc.sync.dma_start(tkb_t[:], wd[f"tkb{l}"][:])
            toutw_row = wp.tile([1, HD], FP32, tag="toutwr")
            nc.sync.dma_start(toutw_row[:], wd[f"toutw{l}"][:])
            toutw_bc = wp.tile([P, HD], FP32, tag="toutwbc")
            nc.gpsimd.partition_broadcast(toutw_bc[:], toutw_row[:])
            toutb_row = wp.tile([1, 1], FP32, tag="toutbr")
            nc.sync.dma_start(toutb_row[:], wd[f"toutb{l}"][:])
            toutb_bc = wp.tile([P, 1], FP32, tag="toutbbc")
            nc.gpsimd.partition_broadcast(toutb_bc[:], toutb_row[:])
            lt_row = wp.tile([1, H], FP32, tag="ltrow")
            nc.sync.dma_start(lt_row[:], wd[f"logtemp{l}"][:])
            nc.vector.tensor_scalar(out=lt_row[:], in0=lt_row[:], scalar1=-1.0,
                                    scalar2=None, op0=OP.mult)
            nc.scalar.activation(lt_row[:], lt_row[:], AF.Exp)
            nc.vector.tensor_scalar(out=lt_row[:], in0=lt_row[:],
                                    scalar1=float(1.0 / math.sqrt(HD)), scalar2=None,
                                    op0=OP.mult)
            sclrow_bc = wp.tile([P, H], FP32, tag="sclbc")
            nc.gpsimd.partition_broadcast(sclrow_bc[:], lt_row[:])

            # ---- sweep 1: K/V/KT table rows -> ccin
            with tc.tile_pool(name="sw1", bufs=2) as sw1, \
                 tc.tile_pool(name="ps1", bufs=2, space="PSUM") as ps1:
                for i in range(NW):
                    hTl = []
                    for fc in range(4):
                        h_ = sw1.tile([P, P], FP32, tag=f"hTl{fc}", name=f"hTl{fc}")
                        nc.sync.dma_start(h_[:], hT_d[fc * P:(fc + 1) * P,
                                                      i * P:(i + 1) * P])
                        hTl.append(h_)
                    mTl = []
                    for fc in range(4):
                        m_ = sw1.tile([P, P], FP32, tag=f"mTl{fc}", name=f"mTl{fc}")
                        nc.sync.dma_start(m_[:], mT_d[fc * P:(fc + 1) * P,
                                                      i * P:(i + 1) * P])
                        mTl.append(m_)
                    kvp = ps1.tile([P, 2 * KVD], FP32, space="PSUM", tag="kvp")
                    for fc in range(4):
                        nc.tensor.matmul(kvp[:], lhsT=hTl[fc][:], rhs=qkvw_t[fc][:, D:],
                                         start=(fc == 0), stop=False)
                    nc.tensor.matmul(kvp[:], lhsT=ones_row[:1, :], rhs=qkvb_t[:1, D:],
                                     start=False, stop=True)
                    row = sw1.tile([P, D + 2 * KVD], FP32, tag="tabrow")
                    emit_rope(row[:, 0:KVD], kvp[:, 0:KVD], i, KVH, sw1)
                    nc.vector.tensor_copy(row[:, KVD:2 * KVD], kvp[:, KVD:2 * KVD])
                    trp = ps1.tile([P, P], FP32, space="PSUM", tag="trp")
                    nc.tensor.transpose(trp[:], row[:, 0:KVD], ident[:])
                    krT = sw1.tile([P, P], FP32, tag="krT")
                    nc.vector.tensor_copy(krT[:], trp[:])
                    for h in range(H):
                        C = sw1.tile([P, P], FP32, tag="ktc")
                        nc.vector.tensor_copy(
                            C[0:HD, :],
                            mTl[h // 2][(h % 2) * HD:(h % 2) * HD + HD, :])
                        nc.vector.tensor_copy(
                            C[HD:2 * HD, :],
                            krT[(h // G) * HD:(h // G) * HD + HD, :])
                        ktp = ps1.tile([P, HD], FP32, space="PSUM", tag="ktp")
                        nc.tensor.matmul(ktp[:], lhsT=C[:], rhs=tkw_t[:], start=True,
                                         stop=False)
                        nc.tensor.matmul(ktp[:], lhsT=ones_row[:1, :], rhs=tkb_t[:1, :],
                                         start=False, stop=True)
                        nc.vector.tensor_copy(
                            row[:, 2 * KVD + h * HD:2 * KVD + (h + 1) * HD], ktp[:])
                    nc.sync.dma_start(ccin_d[i * P:(i + 1) * P, :], row[:])

            # ---- AllGather the table
            nc.gpsimd.collective_compute(
                "AllGather", OP.bypass, replica_groups=[list(range(NC))],
                ins=[ccin_d[:]], outs=[ccout_d[:]])

            # ---- EET = [Ee | ET] per slot
            with tc.tile_pool(name="eetp", bufs=2) as eetp, \
                 tc.tile_pool(name="ps2", bufs=2, space="PSUM") as ps2:
                eetw_t = []
                for fc in range(4):
                    et_ = eetp.tile([P, 2 * KVD], FP32, tag=f"eetw{fc}",
                                    name=f"eetw{fc}", bufs=1)
                    nc.sync.dma_start(et_[:], wd[f"eetw{l}"][fc * P:(fc + 1) * P, :])
                    eetw_t.append(et_)
                eetb_t = eetp.tile([1, 2 * KVD], FP32, tag="eetb", bufs=1)
                nc.sync.dma_start(eetb_t[:], wd[f"eetb{l}"][:])
                for st in range(NT):
                    ep = ps2.tile([P, 2 * KVD], FP32, space="PSUM", tag="eetps")
                    for fc in range(4):
                        ebk = eetp.tile([P, P], FP32, tag="ehtb")
                        nc.sync.dma_start(
                            ebk[:], EHT_d[(fc * NT + st) * P:(fc * NT + st + 1) * P, :])
                        nc.tensor.matmul(ep[:], lhsT=ebk[:], rhs=eetw_t[fc][:],
                                         start=(fc == 0), stop=False)
                    nc.tensor.matmul(ep[:], lhsT=ones_row[:1, :], rhs=eetb_t[:1, :],
                                     start=False, stop=True)
                    ees = eetp.tile([P, 2 * KVD], FP32, tag="eesb")
                    nc.vector.tensor_copy(ees[:], ep[:])
                    nc.sync.dma_start(EET_d[st * P:(st + 1) * P, :], ees[:])

            # ---- edge phase
            with tc.tile_pool(name="gp", bufs=2) as gp, \
                 tc.tile_pool(name="ps3", bufs=2, space="PSUM") as ps3, \
                 tc.tile_pool(name="stv", bufs=1) as stv, \
                 tc.tile_pool(name="sta", bufs=2) as sta, \
                 tc.tile_pool(name="edp", bufs=2) as edp:
                for k in range(NW):
                    hTl = []
                    for fc in range(4):
                        h_ = edp.tile([P, P], FP32, tag=f"ehTl{fc}", name=f"ehTl{fc}")
                        nc.sync.dma_start(h_[:], hT_d[fc * P:(fc + 1) * P,
                                                      k * P:(k + 1) * P])
                        hTl.append(h_)
                    qp = ps3.tile([P, D], FP32, space="PSUM", tag="qp")
                    for fc in range(4):
                        nc.tensor.matmul(qp[:], lhsT=hTl[fc][:], rhs=qkvw_t[fc][:, 0:D],
                                         start=(fc == 0), stop=False)
                    nc.tensor.matmul(qp[:], lhsT=ones_row[:1, :], rhs=qkvb_t[:1, 0:D],
                                     start=False, stop=True)
                    qwin = edp.tile([P, D], FP32, tag="qwin")
                    emit_rope(qwin[:], qp[:], k, H, edp)
                    qrTs = []
                    for qc in range(4):
                        trq = ps3.tile([P, P], FP32, space="PSUM", tag="trq")
                        nc.tensor.transpose(trq[:], qwin[:, qc * P:(qc + 1) * P],
                                            ident[:])
                        qrT = edp.tile([P, P], FP32, tag=f"qrT{qc}", name=f"qrT{qc}",
                                       bufs=1)
                        nc.vector.tensor_copy(qrT[:], trq[:])
                        qrTs.append(qrT)
                    qtwin = edp.tile([P, D], FP32, tag="qtwin")
                    for h in range(H):
                        mTl = edp.tile([HD, P], FP32, tag="emTl", bufs=2)
                        nc.sync.dma_start(
                            mTl[:], mT_d[h * HD:(h + 1) * HD, k * P:(k + 1) * P])
                        C = edp.tile([P, P], FP32, tag="qtc", bufs=1)
                        nc.vector.tensor_copy(C[0:HD, :], mTl[:])
                        nc.vector.tensor_copy(
                            C[HD:2 * HD, :],
                            qrTs[h // 2][(h % 2) * HD:(h % 2) * HD + HD, :])
                        qtp = ps3.tile([P, HD], FP32, space="PSUM", tag="qtp")
                        nc.tensor.matmul(qtp[:], lhsT=C[:], rhs=tqw_t[:], start=True,
                                         stop=False)
                        nc.tensor.matmul(qtp[:], lhsT=ones_row[:1, :], rhs=tqb_t[:1, :],
                                         start=False, stop=True)
                        nc.vector.tensor_copy(qtwin[:, h * HD:(h + 1) * HD], qtp[:])

                    mwin = edp.tile([P, H], FP32, tag="mwin")
                    nc.gpsimd.memset(mwin[:], -1e38)
                    swin = edp.tile([P, H], FP32, tag="swin")
                    nc.gpsimd.memset(swin[:], 0.0)
                    owin = edp.tile([P, D], FP32, tag="owin")
                    nc.gpsimd.memset(owin[:], 0.0)
                    atts, vsts, msts = [], [], []
                    for t in range(That[k]):
                        s0 = (TB[k] + t) * P
                        idx = gp.tile([P, 1], I32, tag="gidx")
                        nc.sync.dma_start(idx[:], srcid_d[s0:s0 + P, :])
                        gt = gp.tile([P, D + 2 * KVD], FP32, tag="gt")
                        nc.gpsimd.indirect_dma_start(
                            out=gt[:], out_offset=None, in_=ccout_d[:],
                            in_offset=bass.IndirectOffsetOnAxis(ap=idx[:, :1], axis=0))
                        eet = gp.tile([P, 2 * KVD], FP32, tag="geet")
                        nc.sync.dma_start(eet[:], EET_d[s0:s0 + P, :])
                        msk = sta.tile([P, 2], FP32, tag=f"msk{t}", name=f"msk{t}")
                        nc.sync.dma_start(msk[:], maskcat_d[s0:s0 + P, :])
                        kj = gp.tile([P, KVD], FP32, tag="kj")
                        nc.vector.tensor_add(kj[:], gt[:, 0:KVD], eet[:, 0:KVD])
                        kt = gp.tile([P, D], FP32, tag="kt")
                        nc.vector.tensor_tensor(out=kgd(kt[:]),
                                                in0=kgd(gt[:, 2 * KVD:]),
                                                in1=kvd(eet[:, KVD:2 * KVD]),
                                                op=OP.add)
                        nc.vector.tensor_mul(kt[:], qtwin[:], kt[:])
                        nc.scalar.activation(kt[:], kt[:], AF.Tanh)
                        nc.vector.tensor_tensor(
                            out=hdv(kt[:]), in0=hdv(kt[:]),
                            in1=toutw_bc[:].unsqueeze(1).to_broadcast([P, H, HD]),
                            op=OP.mult)
                        tl = gp.tile([P, H], FP32, tag="tl")
                        nc.vector.tensor_reduce(tl[:], hdv(kt[:]), axis=X, op=OP.add)
                        trust = gp.tile([P, H], FP32, tag="trust")
                        nc.scalar.activation(trust[:], tl[:], AF.Sigmoid,
                                             bias=toutb_bc[:, :1])
                        qk = gp.tile([P, D], FP32, tag="qk")
                        nc.vector.tensor_tensor(out=kgd(qk[:]), in0=kgd(qwin[:]),
                                                in1=kvd(kj[:]), op=OP.mult)
                        at = sta.tile([P, H], FP32, tag=f"att{t}", name=f"att{t}")
                        nc.vector.tensor_reduce(at[:], hdv(qk[:]), axis=X, op=OP.add)
                        nc.vector.tensor_mul(at[:], at[:], sclrow_bc[:])
                        nc.vector.tensor_mul(at[:], at[:], trust[:])
                        nc.vector.tensor_scalar(out=at[:], in0=at[:],
                                                scalar1=msk[:, 0:1], scalar2=None,
                                                op0=OP.add)
                        nc.vector.tensor_max(mwin[:], mwin[:], at[:])
                        vst = stv.tile([P, KVD], FP32, tag=f"vst{t}", name=f"vst{t}")
                        nc.vector.tensor_copy(vst[:], gt[:, KVD:2 * KVD])
                        atts.append(at); vsts.append(vst); msts.append(msk)
                    for t in range(That[k]):
                        ex = gp.tile([P, H], FP32, tag="ex")
                        nc.vector.tensor_sub(ex[:], atts[t][:], mwin[:])
                        nc.scalar.activation(ex[:], ex[:], AF.Exp)
                        nc.vector.tensor_scalar(out=ex[:], in0=ex[:],
                                                scalar1=msts[t][:, 1:2], scalar2=None,
                                                op0=OP.mult)
                        nc.vector.tensor_add(swin[:], swin[:], ex[:])
                        exv = gp.tile([P, D], FP32, tag="exv")
                        nc.vector.tensor_tensor(
                            out=kgd(exv[:]), in0=kvd(vsts[t][:]),
                            in1=ex[:].rearrange("p (kv g) -> p kv g", kv=KVH)
                                .unsqueeze(3).to_broadcast([P, KVH, G, HD]),
                            op=OP.mult)
                        nc.vector.tensor_add(owin[:], owin[:], exv[:])
                    nc.vector.tensor_scalar(out=swin[:], in0=swin[:], scalar1=1e-30,
                                            scalar2=None, op0=OP.max)
                    rs = gp.tile([P, H], FP32, tag="rs")
                    nc.vector.reciprocal(rs[:], swin[:])
                    anorm = gp.tile([P, D], FP32, tag="anorm")
                    nc.vector.tensor_tensor(
                        out=hdv(anorm[:]), in0=hdv(owin[:]),
                        in1=rs[:].unsqueeze(2).to_broadcast([P, H, HD]), op=OP.mult)
                    for qc in range(4):
                        tra = ps3.tile([P, P], FP32, space="PSUM", tag="tra")
                        nc.tensor.transpose(tra[:], anorm[:, qc * P:(qc + 1) * P],
                                            ident[:])
                        nc.vector.tensor_copy(aT[qc][:, k * P:(k + 1) * P], tra[:])

            # ---------------- node phase B
            with tc.tile_pool(name="nwp", bufs=2) as nwp, \
                 tc.tile_pool(name="nrp", bufs=1) as nrp:
                gatew_t = nwp.tile([P, 4 * NEXP], FP32, tag="gatew", bufs=1)
                for fc in range(4):
                    nc.sync.dma_start(gatew_t[:, fc * NEXP:(fc + 1) * NEXP],
                                      wd[f"gatew{l}"][fc * P:(fc + 1) * P, :])
                gateb_t = nwp.tile([1, NEXP], FP32, tag="gateb", bufs=1)
                nc.sync.dma_start(gateb_t[:], wd[f"gateb{l}"][:])
                ow_t = []
                for fc in range(4):
                    o_ = nwp.tile([P, D], WDT, tag=f"ow{fc}", name=f"ow{fc}", bufs=1)
                    nc.sync.dma_start(o_[:], wd[f"ow{l}"][fc * P:(fc + 1) * P, :])
                    ow_t.append(o_)
                obc_t = nwp.tile([P, 4], FP32, tag="obc", bufs=1)
                nc.sync.dma_start(obc_t[:],
                                  wd[f"obT{l}"].rearrange("(c p) one -> p (c one)", p=P))
                adaM_t = nwp.tile([2, D], FP32, tag="adaM", bufs=1)
                nc.sync.dma_start(adaM_t[:], wd[f"adaM{l}"][:])
                adaA_t = nwp.tile([4, D], FP32, tag="adaA", bufs=1)
                nc.sync.dma_start(adaA_t[:], wd[f"adaA{l}"][:])
                rgw_t = nwp.tile([P, 4], FP32, tag="rgw", bufs=1)
                nc.sync.dma_start(rgw_t[:],
                                  wd[f"rgw{l}"].rearrange("(c p) one -> p (c one)", p=P))
                rgb_t = nwp.tile([1, 1], FP32, tag="rgb", bufs=1)
                nc.sync.dma_start(rgb_t[:], wd[f"rgb{l}"][:])

                with tc.tile_pool(name="psg", bufs=2, space="PSUM") as psg:
                    for i in range(NW):
                        gps = psg.tile([P, NEXP], FP32, space="PSUM", tag="gps")
                        for fc in range(4):
                            nc.tensor.matmul(gps[:], lhsT=aT[fc][:, i * P:(i + 1) * P],
                                             rhs=gatew_t[:, fc * NEXP:(fc + 1) * NEXP],
                                             start=(fc == 0), stop=False)
                        nc.tensor.matmul(gps[:], lhsT=ones_row[:1, :],
                                         rhs=gateb_t[:1, :], start=False, stop=True)
                        lg = nwp.tile([P, NEXP], FP32, tag="lg")
                        nc.vector.tensor_copy(lg[:], gps[:])
                        lgp = nwp.tile([P, NEXP], FP32, tag="lgp")
                        nc.vector.tensor_add(lgp[:], lg[:], idxeps_bc[:])
                        m1 = nwp.tile([P, 1], FP32, tag="m1")
                        nc.vector.tensor_reduce(m1[:], lg[:], axis=X, op=OP.max)
                        m1p = nwp.tile([P, 1], FP32, tag="m1p")
                        nc.vector.tensor_reduce(m1p[:], lgp[:], axis=X, op=OP.max)
                        eq = nwp.tile([P, NEXP], FP32, tag="eq")
                        nc.vector.tensor_scalar(out=eq[:], in0=lgp[:],
                                                scalar1=m1p[:, :1], scalar2=None,
                                                op0=OP.is_equal)
                        nc.vector.scalar_tensor_tensor(out=eq[:], in0=eq[:],
                                                       scalar=-1e38, in1=lgp[:],
                                                       op0=OP.mult, op1=OP.add)
                        m2p = nwp.tile([P, 1], FP32, tag="m2p")
                        nc.vector.tensor_reduce(m2p[:], eq[:], axis=X, op=OP.max)
                        sel = nwp.tile([P, NEXP], FP32, tag="sel")
                        nc.vector.tensor_scalar(out=sel[:], in0=lgp[:],
                                                scalar1=m2p[:, :1], scalar2=None,
                                                op0=OP.is_ge)
                        tmp = nwp.tile([P, NEXP], FP32, tag="gtmp")
                        nc.vector.tensor_mul(tmp[:], lg[:], sel[:])
                        m2o = nwp.tile([P, 1], FP32, tag="m2o")
                        nc.vector.tensor_reduce(m2o[:], tmp[:], axis=X, op=OP.add)
                        nc.vector.tensor_sub(m2o[:], m2o[:], m1[:])
                        dn = nwp.tile([P, 1], FP32, tag="dn")
                        nc.vector.tensor_sub(dn[:], m2o[:], m1[:])
                        nc.scalar.activation(dn[:], dn[:], AF.Exp)
                        nc.vector.tensor_scalar(out=dn[:], in0=dn[:], scalar1=1.0,
                                                scalar2=None, op0=OP.add)
                        rdn = nwp.tile([P, 1], FP32, tag="rdn")
                        nc.vector.reciprocal(rdn[:], dn[:])
                        nc.vector.tensor_scalar(out=tmp[:], in0=lg[:],
                                                scalar1=m1[:, :1], scalar2=None,
                                                op0=OP.subtract)
                        nc.scalar.activation(tmp[:], tmp[:], AF.Exp)
                        nc.vector.tensor_mul(tmp[:], tmp[:], sel[:])
                        nc.vector.tensor_scalar(out=tmp[:], in0=tmp[:],
                                                scalar1=rdn[:, :1], scalar2=None,
                                                op0=OP.mult)
                        for e in range(NEXP):
                            trw = psg.tile([1, P], FP32, space="PSUM", tag="trw")
                            nc.tensor.matmul(trw[:], lhsT=tmp[:, e:e + 1], rhs=ident[:],
                                             start=True, stop=True)
                            wkr = nwp.tile([1, P], FP32, tag="wkr")
                            nc.vector.tensor_copy(wkr[:], trw[:])
                            nc.sync.dma_start(wkT_d[e:e + 1, i * P:(i + 1) * P], wkr[:])

                # MoE + o + AdaLN + residual, per 512-node chunk
                with tc.tile_pool(name="psm", bufs=4, space="PSUM") as psm, \
                     tc.tile_pool(name="psw", bufs=1, space="PSUM") as psw:
                    for nh in range(NPC // 512):
                        nsl = slice(nh * 512, (nh + 1) * 512)
                        moe = [nrp.tile([P, 512], FP32, tag=f"moe{fc}",
                                        name=f"moe{fc}") for fc in range(4)]
                        for fc in range(4):
                            nc.gpsimd.memset(moe[fc][:], 0.0)
                        if bf:
                            rhs_a = []
                            for fc in range(4):
                                ab_ = nwp.tile([P, 512], BF16, tag=f"abf{fc}",
                                               name=f"abf{fc}", bufs=1)
                                nc.vector.tensor_copy(ab_[:], aT[fc][:, nsl])
                                rhs_a.append(ab_[:])
                        else:
                            rhs_a = [aT[fc][:, nsl] for fc in range(4)]
                        for e in range(NEXP):
                            eb1c = nwp.tile([P, 8], FP32, tag="eb1c")
                            nc.sync.dma_start(
                                eb1c[:],
                                wd[f"eb1T_{l}_{e}"].rearrange("(c p) one -> p (c one)",
                                                              p=P))
                            eb2c = nwp.tile([P, 4], FP32, tag="eb2c")
                            nc.sync.dma_start(
                                eb2c[:],
                                wd[f"eb2T_{l}_{e}"].rearrange("(c p) one -> p (c one)",
                                                              p=P))
                            wkrow = nwp.tile([1, 512], FP32, tag="wkrow")
                            nc.sync.dma_start(wkrow[:], wkT_d[e:e + 1, nsl])
                            wkbp = psw.tile([P, 512], FP32, space="PSUM", tag="wkb")
                            nc.tensor.matmul(wkbp[:], lhsT=ones_row[:1, :],
                                             rhs=wkrow[:1, :], start=True, stop=True)
                            wkb = nwp.tile([P, 512], FP32, tag="wkbs", bufs=1)
                            nc.vector.tensor_copy(wkb[:], wkbp[:])
                            hdn = []
                            for oc in range(8):
                                hp = psm.tile([P, 512], FP32, space="PSUM", tag="big")
                                for fc in range(4):
                                    wt = nwp.tile([P, P], WDT, tag=f"w1t{fc}",
                                                  name=f"w1t{fc}")
                                    nc.sync.dma_start(
                                        wt[:], wd[f"ew1_{l}_{e}"][fc * 8 + oc, :, :])
                                    nc.tensor.matmul(hp[:], lhsT=wt[:], rhs=rhs_a[fc],
                                                     start=(fc == 0), stop=(fc == 3))
                                hs = nwp.tile([P, 512], WDT, tag=f"hdn{oc}",
                                              name=f"hdn{oc}", bufs=1)
                                nc.scalar.activation(hs[:], hp[:], AF.Gelu,
                                                     bias=eb1c[:, oc:oc + 1])
                                hdn.append(hs)
                            for oc2 in range(4):
                                ep = psm.tile([P, 512], FP32, space="PSUM", tag="big")
                                for hc in range(8):
                                    wt2 = nwp.tile([P, P], WDT, tag=f"w2t{hc % 4}",
                                                   name=f"w2t{hc % 4}")
                                    nc.sync.dma_start(
                                        wt2[:], wd[f"ew2_{l}_{e}"][hc * 4 + oc2, :, :])
                                    nc.tensor.matmul(ep[:], lhsT=wt2[:], rhs=hdn[hc][:],
                                                     start=(hc == 0), stop=(hc == 7))
                                tmp2 = nwp.tile([P, 512], FP32, tag="eotmp")
                                nc.vector.scalar_tensor_tensor(
                                    out=tmp2[:], in0=ep[:], scalar=eb2c[:, oc2:oc2 + 1],
                                    in1=wkb[:], op0=OP.add, op1=OP.mult)
                                nc.vector.tensor_add(moe[oc2][:], moe[oc2][:], tmp2[:])

                        # ---- o projection + AdaLN + residual for this chunk
                        if bf:
                            mrhs = []
                            for fc in range(4):
                                mb = nwp.tile([P, 512], BF16, tag=f"mbf{fc}",
                                              name=f"mbf{fc}", bufs=1)
                                nc.vector.tensor_copy(mb[:], moe[fc][:])
                                mrhs.append(mb[:])
                        else:
                            mrhs = [moe[fc][:] for fc in range(4)]
                        osb = []
                        for oc in range(4):
                            op_ = psm.tile([P, 512], FP32, space="PSUM", tag="big")
                            for fc in range(4):
                                nc.tensor.matmul(op_[:],
                                                 lhsT=ow_t[fc][:, oc * P:(oc + 1) * P],
                                                 rhs=mrhs[fc], start=(fc == 0),
                                                 stop=(fc == 3))
                            ot = nwp.tile([P, 512], FP32, tag=f"osb{oc}",
                                          name=f"osb{oc}", bufs=1)
                            nc.vector.tensor_scalar(out=ot[:], in0=op_[:],
                                                    scalar1=obc_t[:, oc:oc + 1],
                                                    scalar2=None, op0=OP.add)
                            osb.append(ot)
                        mu_ps = psw.tile([1, 512], FP32, space="PSUM", tag="rowp", bufs=3)
                        for oc in range(4):
                            nc.tensor.matmul(mu_ps[:], lhsT=ones_col[:], rhs=osb[oc][:],
                                             start=(oc == 0), stop=(oc == 3))
                        s2_ps = psw.tile([1, 512], FP32, space="PSUM", tag="rowp", bufs=3)
                        for oc in range(4):
                            sqt = nwp.tile([P, 512], FP32, tag="sqt")
                            nc.scalar.activation(sqt[:], osb[oc][:], AF.Square)
                            nc.tensor.matmul(s2_ps[:], lhsT=ones_col[:], rhs=sqt[:],
                                             start=(oc == 0), stop=(oc == 3))
                        mu = nwp.tile([1, 512], FP32, tag="mur", bufs=1)
                        nc.vector.tensor_scalar(out=mu[:], in0=mu_ps[:],
                                                scalar1=1.0 / D, scalar2=None,
                                                op0=OP.mult)
                        var = nwp.tile([1, 512], FP32, tag="varr", bufs=1)
                        nc.vector.tensor_scalar(out=var[:], in0=s2_ps[:],
                                                scalar1=1.0 / D, scalar2=None,
                                                op0=OP.mult)
                        mu2 = nwp.tile([1, 512], FP32, tag="mu2r", bufs=1)
                        nc.vector.tensor_mul(mu2[:], mu[:], mu[:])
                        nc.vector.tensor_sub(var[:], var[:], mu2[:])
                        nc.vector.tensor_scalar(out=var[:], in0=var[:], scalar1=0.0,
                                                scalar2=None, op0=OP.max)
                        std = nwp.tile([1, 512], FP32, tag="stdr", bufs=1)
                        nc.scalar.activation(std[:], var[:], AF.Sqrt, bias=epsb[:1, :1])
                        rstd = nwp.tile([1, 512], FP32, tag="rstdr", bufs=1)
                        nc.vector.reciprocal(rstd[:], std[:])
                        rg_ps = psw.tile([1, 512], FP32, space="PSUM", tag="rowp", bufs=3)
                        hTn = []
                        for fc in range(4):
                            hh = nwp.tile([P, 512], FP32, tag=f"hTn{fc}",
                                          name=f"hTn{fc}", bufs=1)
                            nc.sync.dma_start(hh[:], hT_d[fc * P:(fc + 1) * P, nsl])
                            hTn.append(hh)
                        for fc in range(4):
                            nc.tensor.matmul(rg_ps[:], lhsT=rgw_t[:, fc:fc + 1],
                                             rhs=hTn[fc][:], start=(fc == 0),
                                             stop=(fc == 3))
                        beta = nwp.tile([1, 512], FP32, tag="betar", bufs=1)
                        nc.scalar.activation(beta[:], rg_ps[:], AF.Sigmoid,
                                             bias=rgb_t[:1, :1])
                        onem = nwp.tile([1, 512], FP32, tag="onemr", bufs=1)
                        nc.vector.tensor_scalar(out=onem[:], in0=beta[:], scalar1=-1.0,
                                                scalar2=1.0, op0=OP.mult, op1=OP.add)
                        r1 = nwp.tile([1, 512], FP32, tag="r1r", bufs=1)
                        nc.vector.tensor_mul(r1[:], onem[:], rstd[:])
                        stage = nwp.tile([1, 6 * 512], FP32, tag="stage", bufs=1)
                        nc.vector.tensor_copy(stage[:1, 0:512], r1[:])
                        nc.vector.tensor_mul(stage[:1, 512:1024], r1[:],
                                             silu_row[:1, nsl])
                        negmu = nwp.tile([1, 512], FP32, tag="negmu", bufs=1)
                        nc.vector.tensor_scalar(out=negmu[:], in0=mu[:], scalar1=-1.0,
                                                scalar2=None, op0=OP.mult)
                        nc.vector.tensor_mul(stage[:1, 1024:1536], negmu[:],
                                             stage[:1, 0:512])
                        nc.vector.tensor_mul(stage[:1, 1536:2048], negmu[:],
                                             stage[:1, 512:1024])
                        nc.vector.tensor_mul(stage[:1, 2048:2560], onem[:],
                                             silu_row[:1, nsl])
                        nc.vector.tensor_copy(stage[:1, 2560:3072], onem[:])
                        rhs2 = nwp.tile([2, 512], FP32, tag="rhs2", bufs=1)
                        rhs4 = nwp.tile([4, 512], FP32, tag="rhs4", bufs=1)
                        for q in range(2):
                            nc.sync.dma_start(rhs2[q:q + 1, :],
                                              stage[:1, q * 512:(q + 1) * 512])
                        for q in range(4):
                            nc.sync.dma_start(rhs4[q:q + 1, :],
                                              stage[:1, (2 + q) * 512:(3 + q) * 512])
                        bbc = psm.tile([P, 512], FP32, space="PSUM", tag="big")
                        nc.tensor.matmul(bbc[:], lhsT=ones_row[:1, :], rhs=beta[:1, :],
                                         start=True, stop=True)
                        for fc in range(4):
                            mco = psm.tile([P, 512], FP32, space="PSUM", tag="big")
                            nc.tensor.matmul(mco[:],
                                             lhsT=adaM_t[:, fc * P:(fc + 1) * P],
                                             rhs=rhs2[:], start=True, stop=True)
                            aco = psm.tile([P, 512], FP32, space="PSUM", tag="big")
                            nc.tensor.matmul(aco[:],
                                             lhsT=adaA_t[:, fc * P:(fc + 1) * P],
                                             rhs=rhs4[:], start=True, stop=True)
                            t1 = nwp.tile([P, 512], FP32, tag="hnew1")
                            nc.vector.tensor_mul(t1[:], hTn[fc][:], bbc[:])
                            t2 = nwp.tile([P, 512], FP32, tag="hnew2")
                            nc.vector.tensor_mul(t2[:], osb[fc][:], mco[:])
                            nc.vector.tensor_add(t1[:], t1[:], t2[:])
                            nc.vector.tensor_add(t1[:], t1[:], aco[:])
                            nc.sync.dma_start(hT_d[fc * P:(fc + 1) * P, nsl], t1[:])

        # ---------------- final layer norm + output
        with tc.tile_pool(name="fin", bufs=2) as fin, \
             tc.tile_pool(name="psf", bufs=2, space="PSUM") as psf, \
             tc.tile_pool(name="psf1", bufs=1, space="PSUM") as psf1:
            fng_t = fin.tile([P, 4], FP32, tag="fng", bufs=1)
            nc.sync.dma_start(fng_t[:], wd["fngT"].rearrange("(c p) one -> p (c one)", p=P))
            fnb_t = fin.tile([P, 4], FP32, tag="fnb", bufs=1)
            nc.sync.dma_start(fnb_t[:], wd["fnbT"].rearrange("(c p) one -> p (c one)", p=P))
            for nh in range(NPC // 512):
                nsl = slice(nh * 512, (nh + 1) * 512)
                hTn = []
                for fc in range(4):
                    hh = fin.tile([P, 512], FP32, tag=f"fhT{fc}", name=f"fhT{fc}",
                                  bufs=1)
                    nc.sync.dma_start(hh[:], hT_d[fc * P:(fc + 1) * P, nsl])
                    hTn.append(hh)
                mu_ps = psf1.tile([1, 512], FP32, space="PSUM", tag="fmu")
                for fc in range(4):
                    nc.tensor.matmul(mu_ps[:], lhsT=ones_col[:], rhs=hTn[fc][:],
                                     start=(fc == 0), stop=(fc == 3))
                s2_ps = psf1.tile([1, 512], FP32, space="PSUM", tag="fs2")
                for fc in range(4):
                    sq = fin.tile([P, 512], FP32, tag="fsq")
                    nc.scalar.activation(sq[:], hTn[fc][:], AF.Square)
                    nc.tensor.matmul(s2_ps[:], lhsT=ones_col[:], rhs=sq[:],
                                     start=(fc == 0), stop=(fc == 3))
                mu = fin.tile([1, 512], FP32, tag="fmur", bufs=1)
                nc.vector.tensor_scalar(out=mu[:], in0=mu_ps[:], scalar1=1.0 / D,
                                        scalar2=None, op0=OP.mult)
                var = fin.tile([1, 512], FP32, tag="fvarr", bufs=1)
                nc.vector.tensor_scalar(out=var[:], in0=s2_ps[:], scalar1=1.0 / D,
                                        scalar2=None, op0=OP.mult)
                mu2 = fin.tile([1, 512], FP32, tag="fmu2", bufs=1)
                nc.vector.tensor_mul(mu2[:], mu[:], mu[:])
                nc.vector.tensor_sub(var[:], var[:], mu2[:])
                nc.vector.tensor_scalar(out=var[:], in0=var[:], scalar1=0.0,
                                        scalar2=None, op0=OP.max)
                std = fin.tile([1, 512], FP32, tag="fstd", bufs=1)
                nc.scalar.activation(std[:], var[:], AF.Sqrt, bias=epsb[:1, :1])
                rstd = fin.tile([1, 512], FP32, tag="frstd", bufs=1)
                nc.vector.reciprocal(rstd[:], std[:])
                negmu = fin.tile([1, 512], FP32, tag="fnegmu", bufs=1)
                nc.vector.tensor_scalar(out=negmu[:], in0=mu[:], scalar1=-1.0,
                                        scalar2=None, op0=OP.mult)
                nmu_bc = psf.tile([P, 512], FP32, space="PSUM", tag="fnmbc")
                nc.tensor.matmul(nmu_bc[:], lhsT=ones_row[:1, :], rhs=negmu[:1, :],
                                 start=True, stop=True)
                rstd_bc = psf.tile([P, 512], FP32, space="PSUM", tag="frsbc")
                nc.tensor.matmul(rstd_bc[:], lhsT=ones_row[:1, :], rhs=rstd[:1, :],
                                 start=True, stop=True)
                nrm = []
                for fc in range(4):
                    t1 = fin.tile([P, 512], FP32, tag=f"fnrm{fc}", name=f"fnrm{fc}",
                                  bufs=1)
                    nc.vector.tensor_add(t1[:], hTn[fc][:], nmu_bc[:])
                    nc.vector.tensor_mul(t1[:], t1[:], rstd_bc[:])
                    nc.vector.tensor_scalar(out=t1[:], in0=t1[:],
                                            scalar1=fng_t[:, fc:fc + 1], scalar2=None,
                                            op0=OP.mult)
                    nc.vector.tensor_scalar(out=t1[:], in0=t1[:],
                                            scalar1=fnb_t[:, fc:fc + 1], scalar2=None,
                                            op0=OP.add)
                    nrm.append(t1)
                for ii in range(4):
                    onode = fin.tile([P, D], FP32, tag="onode")
                    for fc in range(4):
                        trf = psf.tile([P, P], FP32, space="PSUM", tag="ftr")
                        nc.tensor.transpose(trf[:], nrm[fc][:, ii * P:(ii + 1) * P],
                                            ident[:])
                        nc.vector.tensor_copy(onode[:, fc * P:(fc + 1) * P], trf[:])
                    nc.sync.dma_start(
                        outp_d[nh * 512 + ii * P:nh * 512 + (ii + 1) * P, :], onode[:])

    nc.compile()
    return nc, "outp"
